# revision 7
# baseline (speedup 1.0000x reference)
"""Multi-head attention Trainium2 kernel (8 NeuronCores, SPMD).

Sharding: core c handles batch b = c//2 and query-token half c%2 (1024 of
2048 rows). Each core redundantly computes the K/V projections for its
batch (cores sharing a batch compute the same k/v) -- no collectives.

All matmuls run in float32r (TF32-like precision, ~4x faster than fp32 on
the PE). Layout strategy:
  - host pre-tiles every input into the exact SBUF layout so all DMAs are
    contiguous per partition
  - k/q projections produce transposed outputs [out_feat, tok]
  - v projection produces natural [tok, feat] with an interleaved ones
    column per head; k^T and v round-trip through DRAM and are re-streamed
    per head-pair during attention
  - scores computed transposed [k_tok, q_tok]; softmax skips the max
    subtraction (scores are O(5) by construction); exp on ACT with the
    1/sqrt(dk) scale folded in
  - attn @ v computed as v^T @ attn with the ones column producing the
    softmax denominators in psum row 64 (M=65)
  - normalization (reciprocal -> K=1 broadcast matmul -> vector multiply)
    is emitted one head late so the tiny broadcast matmul never blocks the
    in-order PE stream
  - o_proj consumes the accumulated [feat, tok] attention output and writes
    the natural-layout result
"""

import numpy as np

import concourse.bass as bass
import concourse.mybir as mybir
import concourse.tile as tile
from concourse import bacc
from concourse.bass_utils import run_bass_kernel_spmd

F32 = mybir.dt.float32
F32R = mybir.dt.float32r
AF = mybir.ActivationFunctionType

H, DM, DK = 16, 1024, 64
B, L = 4, 2048
TOK = 1024          # query tokens per core
FT = DM // 128      # 8 feature tiles
NCORES = 8
SCALE = 1.0 / np.sqrt(DK)

_cache = {}


def _build_nc():
    nc = bacc.Bacc(None, target_bir_lowering=False)

    # host-pre-tiled inputs (see _make_in_maps for element order)
    qt = nc.dram_tensor("qt", [2, 128, FT, 512], F32R, kind="ExternalInput")
    kt = nc.dram_tensor("kt", [4, 128, FT, 512], F32R, kind="ExternalInput")
    vt = nc.dram_tensor("vt", [16, 128, FT, 128], F32R, kind="ExternalInput")
    wkt = nc.dram_tensor("wkt", [128, FT, DM], F32R, kind="ExternalInput")
    wvt = nc.dram_tensor("wvt", [128, FT, DM], F32R, kind="ExternalInput")
    wqt = nc.dram_tensor("wqt", [128, FT, DM], F32R, kind="ExternalInput")
    wot = nc.dram_tensor("wot", [128, FT, DM], F32R, kind="ExternalInput")
    bk_d = nc.dram_tensor("bk", [DM], F32, kind="ExternalInput")
    bq_d = nc.dram_tensor("bq", [DM], F32, kind="ExternalInput")
    bv_d = nc.dram_tensor("bv", [DM], F32, kind="ExternalInput")
    bo_d = nc.dram_tensor("bo", [DM], F32, kind="ExternalInput")
    out_d = nc.dram_tensor("out", [TOK, DM], F32, kind="ExternalOutput")

    def bcast_ap(dram_ap, parts=128):
        return bass.AP(
            tensor=dram_ap.tensor,
            offset=dram_ap.offset,
            ap=[[0, parts]] + [list(d) for d in dram_ap.ap],
        )

    from contextlib import ExitStack

    with tile.TileContext(nc) as tc, ExitStack() as ctx:
        dram = ctx.enter_context(tc.tile_pool(name="dram", bufs=1, space="DRAM"))
        # kT_dram[h, p, t]: head h's k^T in rows (h%2)*64..+64, other rows zero
        kT_dram = dram.tile([H, 128, L], F32R, name="kT_dram")
        # v_dram[h, p, kt, e]: e 0..63 = v, e 64 = ones, e 65..127 = zeros
        v_dram = dram.tile([H, 128, 16, 128], F32R, name="v_dram")

        persist = ctx.enter_context(tc.tile_pool(name="persist", bufs=1))
        bk_sb = persist.tile([128, FT], F32, name="bk_sb")
        nc.sync.dma_start(bk_sb[:], bk_d[:].rearrange("(f p) -> p f", p=128))
        bq_sb = persist.tile([128, FT], F32, name="bq_sb")
        nc.sync.dma_start(bq_sb[:], bq_d[:].rearrange("(f p) -> p f", p=128))
        bv_rep = persist.tile([128, DM], F32, name="bv_rep")
        nc.gpsimd.dma_start(bv_rep[:], bcast_ap(bv_d[:]))
        bo_rep = persist.tile([128, DM], F32, name="bo_rep")
        nc.gpsimd.dma_start(bo_rep[:], bcast_ap(bo_d[:]))
        ones_f32 = persist.tile([65, DK], F32, name="ones_f32")
        nc.vector.memset(ones_f32[:], 1.0)
        ones_t = persist.tile([65, DK], F32R, name="ones_t")
        nc.vector.tensor_copy(ones_t[:], ones_f32[:])
        ones_col = persist.tile([128, 4, 2, 1], F32, name="ones_col")
        nc.vector.memset(ones_col[:], 1.0)
        zeros_sb = persist.tile([128, L], F32, name="zeros_sb")
        nc.vector.memset(zeros_sb[:], 0.0)

        def emit_pad_zeroing():
            for h in range(H):
                pad0 = (1 - h % 2) * 64  # rows NOT holding head h's data
                nc.sync.dma_start(
                    kT_dram[h, pad0:pad0 + 64, :].bitcast(F32), zeros_sb[0:64, :]
                )
                nc.sync.dma_start(
                    v_dram[h, :, :, 65:128].bitcast(F32),
                    zeros_sb[:, 0:16 * 63].rearrange("p (kt e) -> p kt e", e=63),
                )

        # ---- Phase 1: k-proj (transposed out) and v-proj (natural out) ----
        with (
            tc.tile_pool(name="p1w", bufs=1) as wpool,
            tc.tile_pool(name="p1c", bufs=2) as cpool,
            tc.tile_pool(name="p1s", bufs=3) as spool,
            tc.tile_pool(name="psA", bufs=5, space="PSUM") as psA,
        ):
            wk_sb = wpool.tile([128, FT, DM], F32R, name="wk_sb")
            nc.sync.dma_start(wk_sb[:], wkt[:, :, :])
            wv_sb = wpool.tile([128, FT, DM], F32R, name="wv_sb")
            nc.sync.dma_start(wv_sb[:], wvt[:, :, :])

            # k^T = (Wk^T)^T @ K^T : lhsT = WkT tile, moving = K^T chunk
            ktc = wpool.tile([128, FT, L], F32R, name="ktc")
            for n in range(4):
                nc.sync.dma_start(ktc[:, :, n * 512:(n + 1) * 512], kt[n])
            emit_pad_zeroing()
            for m in range(FT):
                pss = []
                for n in range(4):
                    pss.append(psA.tile([128, 512], F32, name="psA", tag="psA"))
                for k in range(FT):
                    for n in range(4):
                        nc.tensor.matmul(
                            pss[n][:],
                            wk_sb[:, k, m * 128:(m + 1) * 128],
                            ktc[:, k, n * 512:(n + 1) * 512],
                            start=(k == 0),
                            stop=(k == FT - 1),
                        )
                for n in range(4):
                    stg = spool.tile([128, 512], F32R, name="stg")
                    nc.scalar.activation(
                        stg[:], pss[n][:], AF.Identity, bias=bk_sb[:, m:m + 1]
                    )
                    nc.sync.dma_start(
                        kT_dram[2 * m, 0:64, n * 512:(n + 1) * 512], stg[0:64, :]
                    )
                    nc.sync.dma_start(
                        kT_dram[2 * m + 1, 64:128, n * 512:(n + 1) * 512],
                        stg[64:128, :],
                    )

            # v = (V^T)^T @ Wv^T : lhsT = V^T tile, moving = WvT chunk
            # n-outer: heads 0..7 (n=0) finish before heads 8..15 start
            for n in range(2):
                for m in range(16):
                    vtc = cpool.tile([128, FT, 128], F32R, name="vtc")
                    nc.sync.dma_start(vtc[:], vt[m])
                    ps_v = psA.tile([128, 512], F32, name="psAv", tag="psA")
                    for k in range(FT):
                        nc.tensor.matmul(
                            ps_v[:],
                            vtc[:, k, :],
                            wv_sb[:, k, n * 512:(n + 1) * 512],
                            start=(k == 0),
                            stop=(k == FT - 1),
                        )
                    # staging in interleaved per-pair layout [4 pairs, 2 heads, 65]
                    stg = spool.tile([128, 4, 2, 65], F32R, name="stgv")
                    nc.vector.tensor_add(
                        stg[:, :, :, 0:DK],
                        ps_v[:].rearrange("p (pp hh d) -> p pp hh d", pp=4, d=DK),
                        bv_rep[:, n * 512:(n + 1) * 512].rearrange(
                            "p (pp hh d) -> p pp hh d", pp=4, d=DK
                        ),
                    )
                    nc.vector.tensor_copy(stg[:, :, :, DK:DK + 1], ones_col[:])
                    for pp in range(4):
                        for hh in range(2):
                            h = n * 8 + pp * 2 + hh
                            nc.sync.dma_start(
                                v_dram[h, :, m, 0:65], stg[:, pp, hh, :]
                            )

        # ---- Phase 2: q-proj (transposed out, SBUF resident) ----
        qTpool = ctx.enter_context(tc.tile_pool(name="qTp", bufs=1))
        qT_sb = qTpool.tile([128, FT, TOK], F32R, name="qT_sb")
        with (
            tc.tile_pool(name="p3w", bufs=1) as wq_pool,
            tc.tile_pool(name="psB", bufs=5, space="PSUM") as psB,
        ):
            wq_sb = wq_pool.tile([128, FT, DM], F32R, name="wq_sb")
            nc.sync.dma_start(wq_sb[:], wqt[:, :, :])
            qtc = wq_pool.tile([128, FT, TOK], F32R, name="qtc")
            for n in range(2):
                nc.sync.dma_start(qtc[:, :, n * 512:(n + 1) * 512], qt[n])
            for m in range(FT):
                pss = []
                for n in range(2):
                    pss.append(psB.tile([128, 512], F32, name="psB", tag="psB"))
                for k in range(FT):
                    for n in range(2):
                        nc.tensor.matmul(
                            pss[n][:],
                            wq_sb[:, k, m * 128:(m + 1) * 128],
                            qtc[:, k, n * 512:(n + 1) * 512],
                            start=(k == 0),
                            stop=(k == FT - 1),
                        )
                for n in range(2):
                    nc.scalar.activation(
                        qT_sb[:, m, n * 512:(n + 1) * 512],
                        pss[n][:],
                        AF.Identity,
                        bias=bq_sb[:, m:m + 1],
                    )

        # ---- Phase 3: attention, per head, full-array matmuls ----
        aout_pool = ctx.enter_context(tc.tile_pool(name="aout", bufs=1))
        attn_outT = aout_pool.tile([128, FT, TOK], F32R, name="attn_outT")
        with (
            tc.tile_pool(name="kv", bufs=3) as kv_pool,
            tc.tile_pool(name="at", bufs=4) as at_pool,
            tc.tile_pool(name="nrm", bufs=2) as nrm_pool,
            tc.tile_pool(name="stash", bufs=2) as stash_pool,
            tc.tile_pool(name="ps", bufs=4, space="PSUM") as ps_pool,
        ):
            def emit_norm(stash, h):
                recip = nrm_pool.tile([65, TOK], F32R, name="recip")
                with nc.allow_low_precision(reason="softmax denom reciprocal"):
                    nc.vector.reciprocal(recip[64:65, :], stash[64:65, :])
                bc = ps_pool.tile([128, TOK], F32, name="bc", tag="ps")
                for qh in range(2):
                    nc.tensor.matmul(
                        bc[0:64, qh * 512:(qh + 1) * 512],
                        ones_t[64:65, :],
                        recip[64:65, qh * 512:(qh + 1) * 512],
                        start=True,
                        stop=True,
                    )
                bc_sb = nrm_pool.tile([64, TOK], F32, name="bc_sb")
                nc.vector.tensor_copy(bc_sb[:], bc[0:64, :])
                pair, hh = divmod(h, 2)
                if hh == 0:
                    nc.vector.tensor_mul(
                        attn_outT[0:64, pair, :], stash[0:64, :], bc_sb[:]
                    )
                else:
                    tmp = nrm_pool.tile([64, TOK], F32R, name="tmp")
                    nc.vector.tensor_mul(tmp[:], stash[0:64, :], bc_sb[:])
                    nc.sync.dma_start(attn_outT[64:128, pair, :], tmp[:])

            pending = None
            for h in range(H):
                kT_head = kv_pool.tile([128, L], F32R, name="kT_head")
                nc.sync.dma_start(kT_head[:], kT_dram[h])
                v_head = kv_pool.tile([128, 16, 128], F32R, name="v_head")
                nc.sync.dma_start(v_head[:], v_dram[h])
                av = ps_pool.tile([128, TOK], F32, name="av", tag="ps")
                for kt_i in range(16):
                    sc = ps_pool.tile([128, TOK], F32, name="sc", tag="ps")
                    for qh in range(2):
                        nc.tensor.matmul(
                            sc[:, qh * 512:(qh + 1) * 512],
                            kT_head[:, kt_i * 128:(kt_i + 1) * 128],
                            qT_sb[:, h // 2, qh * 512:(qh + 1) * 512],
                            start=True,
                            stop=True,
                        )
                    atn = at_pool.tile([128, TOK], F32R, name="atn")
                    nc.scalar.activation(atn[:], sc[:], AF.Exp, scale=SCALE)
                    for qh in range(2):
                        nc.tensor.matmul(
                            av[:, qh * 512:(qh + 1) * 512],
                            v_head[:, kt_i, :],
                            atn[:, qh * 512:(qh + 1) * 512],
                            start=(kt_i == 0),
                            stop=(kt_i == 15),
                        )
                    if kt_i == 6 and pending is not None:
                        emit_norm(*pending)
                        pending = None
                stash = stash_pool.tile([65, TOK], F32, name="stash")
                nc.vector.tensor_copy(stash[:], av[0:65, :])
                pending = (stash, h)
            emit_norm(*pending)

        # ---- Phase 4: o-proj ----
        with (
            tc.tile_pool(name="p5w", bufs=1) as wo_pool,
            tc.tile_pool(name="p5o", bufs=3) as o_pool,
            tc.tile_pool(name="psC", bufs=5, space="PSUM") as psC,
        ):
            wo_sb = wo_pool.tile([128, FT, DM], F32R, name="wo_sb")
            nc.sync.dma_start(wo_sb[:], wot[:, :, :])
            for m in range(FT):
                pss = []
                for n in range(2):
                    pss.append(psC.tile([128, 512], F32, name="psC", tag="psC"))
                for k in range(FT):
                    for n in range(2):
                        nc.tensor.matmul(
                            pss[n][:],
                            attn_outT[:, k, m * 128:(m + 1) * 128],
                            wo_sb[:, k, n * 512:(n + 1) * 512],
                            start=(k == 0),
                            stop=(k == FT - 1),
                        )
                osb = o_pool.tile([128, DM], F32, name="osb")
                for n in range(2):
                    nc.vector.tensor_add(
                        osb[:, n * 512:(n + 1) * 512],
                        pss[n][:],
                        bo_rep[:, n * 512:(n + 1) * 512],
                    )
                nc.sync.dma_start(out_d[m * 128:(m + 1) * 128, :], osb[:])

    nc.compile()
    return nc


def _get_nc():
    if "nc" not in _cache:
        _cache["nc"] = _build_nc()
    return _cache["nc"]


def _tile_w(WT):
    # [in, out] -> [128, FT, out] with element [p, f, o] = WT[f*128+p, o]
    return np.ascontiguousarray(WT.reshape(FT, 128, DM).transpose(1, 0, 2))


def _tile_x(XT, n_chunks, chunk):
    # XT [feat, tok] -> [n_chunks, 128, FT, chunk], [n,p,f,t] = XT[f*128+p, n*chunk+t]
    ntok = n_chunks * chunk
    assert XT.shape == (DM, ntok)
    return np.ascontiguousarray(
        XT.reshape(FT, 128, n_chunks, chunk).transpose(2, 1, 0, 3)
    )


def _make_in_maps(V, K, Q, Wv, bv, Wk, bk, Wq, bq, Wo, bo):
    f32 = np.float32
    WqT = _tile_w(np.asarray(Wq, f32).T)
    WkT = _tile_w(np.asarray(Wk, f32).T)
    WvT = _tile_w(np.asarray(Wv, f32).T)
    WoT = _tile_w(np.asarray(Wo, f32).T)
    bq = np.ascontiguousarray(bq, dtype=f32)
    bk = np.ascontiguousarray(bk, dtype=f32)
    bv = np.ascontiguousarray(bv, dtype=f32)
    bo = np.ascontiguousarray(bo, dtype=f32)
    KTs = [_tile_x(np.asarray(K[b], f32).T, 4, 512) for b in range(B)]
    VTs = [_tile_x(np.asarray(V[b], f32).T, 16, 128) for b in range(B)]
    in_maps = []
    for c in range(NCORES):
        b, half = divmod(c, 2)
        qt = _tile_x(
            np.ascontiguousarray(np.asarray(Q[b], f32)[half * TOK:(half + 1) * TOK].T),
            2, 512,
        )
        in_maps.append({
            "qt": qt, "kt": KTs[b], "vt": VTs[b],
            "wqt": WqT, "wkt": WkT, "wvt": WvT, "wot": WoT,
            "bq": bq, "bk": bk, "bv": bv, "bo": bo,
        })
    return in_maps


def _run(V, K, Q, Wv, bv, Wk, bk, Wq, bq, Wo, bo, **run_kwargs):
    nc = _get_nc()
    in_maps = _make_in_maps(V, K, Q, Wv, bv, Wk, bk, Wq, bq, Wo, bo)
    res = run_bass_kernel_spmd(nc, in_maps, core_ids=list(range(NCORES)), **run_kwargs)
    out = np.empty((B, L, DM), np.float32)
    for c, r in enumerate(res.results):
        b, half = divmod(c, 2)
        out[b, half * TOK:(half + 1) * TOK, :] = r["out"]
    return out, res


def kernel(V, K, Q, Wv, bv, Wk, bk, Wq, bq, Wo, bo):
    out, _ = _run(V, K, Q, Wv, bv, Wk, bk, Wq, bq, Wo, bo)
    return out


# revision 9
# speedup vs baseline: 1.3464x; 1.3464x over previous
"""Multi-head attention Trainium2 kernel (8 NeuronCores, SPMD).

Sharding: core c handles batch b = c//2 and query-token half c%2 (1024 of
2048 rows). Each core redundantly computes the K/V projections for its
batch (cores sharing a batch compute the same k/v) -- no collectives.

All matmuls run in float32r (TF32-like precision, ~4x faster than fp32 on
the PE). Layout strategy:
  - host pre-tiles every input into the exact SBUF layout so all DMAs are
    contiguous per partition
  - k/q projections produce transposed outputs [out_feat, tok]
  - v projection produces natural [tok, feat] with an interleaved ones
    column per head; k^T and v round-trip through DRAM and are re-streamed
    per head-pair during attention
  - scores computed transposed [k_tok, q_tok]; softmax skips the max
    subtraction (scores are O(5) by construction); exp on ACT with the
    1/sqrt(dk) scale folded in
  - attn @ v computed as v^T @ attn with the ones column producing the
    softmax denominators in psum row 64 (M=65)
  - normalization (reciprocal -> K=1 broadcast matmul -> vector multiply)
    is emitted one head late so the tiny broadcast matmul never blocks the
    in-order PE stream
  - o_proj consumes the accumulated [feat, tok] attention output and writes
    the natural-layout result
"""

import numpy as np

import concourse.bass as bass
import concourse.mybir as mybir
import concourse.tile as tile
from concourse import bacc
from concourse.bass_utils import run_bass_kernel_spmd

F32 = mybir.dt.float32
F32R = mybir.dt.float32r
AF = mybir.ActivationFunctionType

H, DM, DK = 16, 1024, 64
B, L = 4, 2048
TOK = 1024          # query tokens per core
FT = DM // 128      # 8 feature tiles
NCORES = 8
SCALE = 1.0 / np.sqrt(DK)

_cache = {}


def _build_nc():
    nc = bacc.Bacc(None, target_bir_lowering=False)

    # host-pre-tiled inputs (see _make_in_maps for element order)
    qt = nc.dram_tensor("qt", [2, 128, FT, 512], F32R, kind="ExternalInput")
    kt = nc.dram_tensor("kt", [4, 128, FT, 512], F32R, kind="ExternalInput")
    vt = nc.dram_tensor("vt", [16, 128, FT, 128], F32R, kind="ExternalInput")
    wkt = nc.dram_tensor("wkt", [128, FT, DM], F32R, kind="ExternalInput")
    wvt = nc.dram_tensor("wvt", [128, FT, DM], F32R, kind="ExternalInput")
    wqt = nc.dram_tensor("wqt", [128, FT, DM], F32R, kind="ExternalInput")
    wot = nc.dram_tensor("wot", [128, FT, DM], F32R, kind="ExternalInput")
    bk_d = nc.dram_tensor("bk", [DM], F32, kind="ExternalInput")
    bq_d = nc.dram_tensor("bq", [DM], F32, kind="ExternalInput")
    bv_d = nc.dram_tensor("bv", [DM], F32, kind="ExternalInput")
    bo_d = nc.dram_tensor("bo", [DM], F32, kind="ExternalInput")
    out_d = nc.dram_tensor("out", [TOK, DM], F32, kind="ExternalOutput")

    def bcast_ap(dram_ap, parts=128):
        return bass.AP(
            tensor=dram_ap.tensor,
            offset=dram_ap.offset,
            ap=[[0, parts]] + [list(d) for d in dram_ap.ap],
        )

    from contextlib import ExitStack

    with tile.TileContext(nc) as tc, ExitStack() as ctx:
        dram = ctx.enter_context(tc.tile_pool(name="dram", bufs=1, space="DRAM"))
        # kT_dram[pair, p, t]: head pair's k^T (two heads stacked on partitions)
        kT_dram = dram.tile([8, 128, L], F32R, name="kT_dram")
        # v_dram[h, p, kt, e]: e 0..63 = v, e 64 = ones
        v_dram = dram.tile([H, 128, 16, 65], F32R, name="v_dram")

        persist = ctx.enter_context(tc.tile_pool(name="persist", bufs=1))
        bk_sb = persist.tile([128, FT], F32, name="bk_sb")
        nc.sync.dma_start(bk_sb[:], bk_d[:].rearrange("(f p) -> p f", p=128))
        bq_sb = persist.tile([128, FT], F32, name="bq_sb")
        nc.sync.dma_start(bq_sb[:], bq_d[:].rearrange("(f p) -> p f", p=128))
        bv_rep = persist.tile([128, DM], F32, name="bv_rep")
        nc.gpsimd.dma_start(bv_rep[:], bcast_ap(bv_d[:]))
        bo_rep = persist.tile([128, DM], F32, name="bo_rep")
        nc.gpsimd.dma_start(bo_rep[:], bcast_ap(bo_d[:]))
        ones_f32 = persist.tile([65, DK], F32, name="ones_f32")
        nc.vector.memset(ones_f32[:], 1.0)
        ones_t = persist.tile([65, DK], F32R, name="ones_t")
        nc.vector.tensor_copy(ones_t[:], ones_f32[:])
        ones_col = persist.tile([128, 4, 2, 1], F32, name="ones_col")
        nc.vector.memset(ones_col[:], 1.0)
        zeros_sb = persist.tile([128, L], F32, name="zeros_sb")
        nc.vector.memset(zeros_sb[:], 0.0)

        # ---- Phase 1: k-proj (transposed out) and v-proj (natural out) ----
        with (
            tc.tile_pool(name="p1w", bufs=1) as wpool,
            tc.tile_pool(name="p1c", bufs=2) as cpool,
            tc.tile_pool(name="p1s", bufs=3) as spool,
            tc.tile_pool(name="psA", bufs=5, space="PSUM") as psA,
        ):
            wk_sb = wpool.tile([128, FT, DM], F32R, name="wk_sb")
            nc.sync.dma_start(wk_sb[:], wkt[:, :, :])
            wv_sb = wpool.tile([128, FT, DM], F32R, name="wv_sb")
            nc.sync.dma_start(wv_sb[:], wvt[:, :, :])

            # k^T = (Wk^T)^T @ K^T : lhsT = WkT tile, moving = K^T chunk
            ktc = wpool.tile([128, FT, L], F32R, name="ktc")
            for n in range(4):
                nc.sync.dma_start(ktc[:, :, n * 512:(n + 1) * 512], kt[n])
            for m in range(FT):
                pss = []
                for n in range(4):
                    pss.append(psA.tile([128, 512], F32, name="psA", tag="psA"))
                for k in range(FT):
                    for n in range(4):
                        nc.tensor.matmul(
                            pss[n][:],
                            wk_sb[:, k, m * 128:(m + 1) * 128],
                            ktc[:, k, n * 512:(n + 1) * 512],
                            start=(k == 0),
                            stop=(k == FT - 1),
                        )
                for n in range(4):
                    stg = spool.tile([128, 512], F32R, name="stg")
                    nc.scalar.activation(
                        stg[:], pss[n][:], AF.Identity, bias=bk_sb[:, m:m + 1]
                    )
                    nc.sync.dma_start(
                        kT_dram[m, :, n * 512:(n + 1) * 512], stg[:]
                    )

            # v = (V^T)^T @ Wv^T : lhsT = V^T tile, moving = WvT chunk
            for m in range(16):
                vtc = cpool.tile([128, FT, 128], F32R, name="vtc")
                nc.sync.dma_start(vtc[:], vt[m])
                pss = []
                for n in range(2):
                    pss.append(psA.tile([128, 512], F32, name="psAv", tag="psA"))
                for k in range(FT):
                    for n in range(2):
                        nc.tensor.matmul(
                            pss[n][:],
                            vtc[:, k, :],
                            wv_sb[:, k, n * 512:(n + 1) * 512],
                            start=(k == 0),
                            stop=(k == FT - 1),
                        )
                for n in range(2):
                    # staging in interleaved per-pair layout [4 pairs, 2 heads, 65]
                    stg = spool.tile([128, 4, 2, 65], F32R, name="stgv")
                    nc.vector.tensor_add(
                        stg[:, :, :, 0:DK],
                        pss[n][:].rearrange("p (pp hh d) -> p pp hh d", pp=4, d=DK),
                        bv_rep[:, n * 512:(n + 1) * 512].rearrange(
                            "p (pp hh d) -> p pp hh d", pp=4, d=DK
                        ),
                    )
                    nc.vector.tensor_copy(stg[:, :, :, DK:DK + 1], ones_col[:])
                    for pp in range(4):
                        for hh in range(2):
                            h = n * 8 + pp * 2 + hh
                            nc.sync.dma_start(
                                v_dram[h, :, m, :], stg[:, pp, hh, :]
                            )

        # ---- Phase 2: q-proj (transposed out, SBUF resident) ----
        qTpool = ctx.enter_context(tc.tile_pool(name="qTp", bufs=1))
        qT_sb = qTpool.tile([128, FT, TOK], F32R, name="qT_sb")
        with (
            tc.tile_pool(name="p3w", bufs=1) as wq_pool,
            tc.tile_pool(name="psB", bufs=5, space="PSUM") as psB,
        ):
            wq_sb = wq_pool.tile([128, FT, DM], F32R, name="wq_sb")
            nc.sync.dma_start(wq_sb[:], wqt[:, :, :])
            qtc = wq_pool.tile([128, FT, TOK], F32R, name="qtc")
            for n in range(2):
                nc.sync.dma_start(qtc[:, :, n * 512:(n + 1) * 512], qt[n])
            for m in range(FT):
                pss = []
                for n in range(2):
                    pss.append(psB.tile([128, 512], F32, name="psB", tag="psB"))
                for k in range(FT):
                    for n in range(2):
                        nc.tensor.matmul(
                            pss[n][:],
                            wq_sb[:, k, m * 128:(m + 1) * 128],
                            qtc[:, k, n * 512:(n + 1) * 512],
                            start=(k == 0),
                            stop=(k == FT - 1),
                        )
                for n in range(2):
                    nc.scalar.activation(
                        qT_sb[:, m, n * 512:(n + 1) * 512],
                        pss[n][:],
                        AF.Identity,
                        bias=bq_sb[:, m:m + 1],
                    )

        # ---- Phase 3: attention, per head, full-array matmuls ----
        aout_pool = ctx.enter_context(tc.tile_pool(name="aout", bufs=1))
        attn_outT = aout_pool.tile([128, FT, TOK], F32R, name="attn_outT")
        with (
            tc.tile_pool(name="kv", bufs=1) as kv_pool,
            tc.tile_pool(name="at", bufs=4) as at_pool,
            tc.tile_pool(name="nrm", bufs=2) as nrm_pool,
            tc.tile_pool(name="stash", bufs=2) as stash_pool,
            tc.tile_pool(name="ps", bufs=4, space="PSUM") as ps_pool,
        ):
            def emit_norm(stash, h):
                recip = nrm_pool.tile([65, TOK], F32R, name="recip")
                with nc.allow_low_precision(reason="softmax denom reciprocal"):
                    nc.vector.reciprocal(recip[64:65, :], stash[64:65, :])
                bc = ps_pool.tile([128, TOK], F32, name="bc", tag="ps")
                for qh in range(2):
                    nc.tensor.matmul(
                        bc[0:64, qh * 512:(qh + 1) * 512],
                        ones_t[64:65, :],
                        recip[64:65, qh * 512:(qh + 1) * 512],
                        start=True,
                        stop=True,
                    )
                bc_sb = nrm_pool.tile([64, TOK], F32, name="bc_sb")
                nc.vector.tensor_copy(bc_sb[:], bc[0:64, :])
                pair, hh = divmod(h, 2)
                if hh == 0:
                    nc.vector.tensor_mul(
                        attn_outT[0:64, pair, :], stash[0:64, :], bc_sb[:]
                    )
                else:
                    tmp = nrm_pool.tile([64, TOK], F32R, name="tmp")
                    nc.vector.tensor_mul(tmp[:], stash[0:64, :], bc_sb[:])
                    nc.sync.dma_start(attn_outT[64:128, pair, :], tmp[:])

            kT_tiles = []
            v_tiles = []
            for par in range(2):
                kt_t = kv_pool.tile([128, L], F32R, name=f"kTp{par}", tag=f"kTp{par}")
                pad0 = (1 - par) * 64
                nc.vector.tensor_copy(kt_t[pad0:pad0 + 64, :], zeros_sb[0:64, :])
                kT_tiles.append(kt_t)
                v_t = kv_pool.tile([128, 16, 128], F32R, name=f"vp{par}", tag=f"vp{par}")
                nc.vector.tensor_copy(
                    v_t[:, :, 65:128],
                    zeros_sb[:, 0:16 * 63].rearrange("p (kt e) -> p kt e", e=63),
                )
                v_tiles.append(v_t)

            pending = None
            for h in range(H):
                par = h % 2
                base = par * 64
                kT_head = kT_tiles[par]
                nc.sync.dma_start(
                    kT_head[base:base + 64, :], kT_dram[h // 2, base:base + 64, :]
                )
                v_head = v_tiles[par]
                nc.sync.dma_start(v_head[:, :, 0:65], v_dram[h])
                av = ps_pool.tile([128, TOK], F32, name="av", tag="ps")
                for kt_i in range(16):
                    sc = ps_pool.tile([128, TOK], F32, name="sc", tag="ps")
                    for qh in range(2):
                        nc.tensor.matmul(
                            sc[:, qh * 512:(qh + 1) * 512],
                            kT_head[:, kt_i * 128:(kt_i + 1) * 128],
                            qT_sb[:, h // 2, qh * 512:(qh + 1) * 512],
                            start=True,
                            stop=True,
                        )
                    atn = at_pool.tile([128, TOK], F32R, name="atn")
                    nc.scalar.activation(atn[:], sc[:], AF.Exp, scale=SCALE)
                    for qh in range(2):
                        nc.tensor.matmul(
                            av[:, qh * 512:(qh + 1) * 512],
                            v_head[:, kt_i, :],
                            atn[:, qh * 512:(qh + 1) * 512],
                            start=(kt_i == 0),
                            stop=(kt_i == 15),
                        )
                    if kt_i == 6 and pending is not None:
                        emit_norm(*pending)
                        pending = None
                stash = stash_pool.tile([65, TOK], F32, name="stash")
                nc.vector.tensor_copy(stash[:], av[0:65, :])
                pending = (stash, h)
            emit_norm(*pending)

        # ---- Phase 4: o-proj ----
        with (
            tc.tile_pool(name="p5w", bufs=1) as wo_pool,
            tc.tile_pool(name="p5o", bufs=3) as o_pool,
            tc.tile_pool(name="psC", bufs=5, space="PSUM") as psC,
        ):
            wo_sb = wo_pool.tile([128, FT, DM], F32R, name="wo_sb")
            nc.sync.dma_start(wo_sb[:], wot[:, :, :])
            for m in range(FT):
                pss = []
                for n in range(2):
                    pss.append(psC.tile([128, 512], F32, name="psC", tag="psC"))
                for k in range(FT):
                    for n in range(2):
                        nc.tensor.matmul(
                            pss[n][:],
                            attn_outT[:, k, m * 128:(m + 1) * 128],
                            wo_sb[:, k, n * 512:(n + 1) * 512],
                            start=(k == 0),
                            stop=(k == FT - 1),
                        )
                osb = o_pool.tile([128, DM], F32, name="osb")
                for n in range(2):
                    nc.vector.tensor_add(
                        osb[:, n * 512:(n + 1) * 512],
                        pss[n][:],
                        bo_rep[:, n * 512:(n + 1) * 512],
                    )
                nc.sync.dma_start(out_d[m * 128:(m + 1) * 128, :], osb[:])

    nc.compile()
    return nc


def _get_nc():
    if "nc" not in _cache:
        _cache["nc"] = _build_nc()
    return _cache["nc"]


def _tile_w(WT):
    # [in, out] -> [128, FT, out] with element [p, f, o] = WT[f*128+p, o]
    return np.ascontiguousarray(WT.reshape(FT, 128, DM).transpose(1, 0, 2))


def _tile_x(XT, n_chunks, chunk):
    # XT [feat, tok] -> [n_chunks, 128, FT, chunk], [n,p,f,t] = XT[f*128+p, n*chunk+t]
    ntok = n_chunks * chunk
    assert XT.shape == (DM, ntok)
    return np.ascontiguousarray(
        XT.reshape(FT, 128, n_chunks, chunk).transpose(2, 1, 0, 3)
    )


def _make_in_maps(V, K, Q, Wv, bv, Wk, bk, Wq, bq, Wo, bo):
    f32 = np.float32
    WqT = _tile_w(np.asarray(Wq, f32).T)
    WkT = _tile_w(np.asarray(Wk, f32).T)
    WvT = _tile_w(np.asarray(Wv, f32).T)
    WoT = _tile_w(np.asarray(Wo, f32).T)
    bq = np.ascontiguousarray(bq, dtype=f32)
    bk = np.ascontiguousarray(bk, dtype=f32)
    bv = np.ascontiguousarray(bv, dtype=f32)
    bo = np.ascontiguousarray(bo, dtype=f32)
    KTs = [_tile_x(np.asarray(K[b], f32).T, 4, 512) for b in range(B)]
    VTs = [_tile_x(np.asarray(V[b], f32).T, 16, 128) for b in range(B)]
    in_maps = []
    for c in range(NCORES):
        b, half = divmod(c, 2)
        qt = _tile_x(
            np.ascontiguousarray(np.asarray(Q[b], f32)[half * TOK:(half + 1) * TOK].T),
            2, 512,
        )
        in_maps.append({
            "qt": qt, "kt": KTs[b], "vt": VTs[b],
            "wqt": WqT, "wkt": WkT, "wvt": WvT, "wot": WoT,
            "bq": bq, "bk": bk, "bv": bv, "bo": bo,
        })
    return in_maps


def _run(V, K, Q, Wv, bv, Wk, bk, Wq, bq, Wo, bo, **run_kwargs):
    nc = _get_nc()
    in_maps = _make_in_maps(V, K, Q, Wv, bv, Wk, bk, Wq, bq, Wo, bo)
    res = run_bass_kernel_spmd(nc, in_maps, core_ids=list(range(NCORES)), **run_kwargs)
    out = np.empty((B, L, DM), np.float32)
    for c, r in enumerate(res.results):
        b, half = divmod(c, 2)
        out[b, half * TOK:(half + 1) * TOK, :] = r["out"]
    return out, res


def kernel(V, K, Q, Wv, bv, Wk, bk, Wq, bq, Wo, bo):
    out, _ = _run(V, K, Q, Wv, bv, Wk, bk, Wq, bq, Wo, bo)
    return out


# revision 12
# speedup vs baseline: 1.4365x; 1.0669x over previous
"""Multi-head attention Trainium2 kernel (8 NeuronCores, SPMD).

Sharding: core c handles batch b = c//2 and query-token half c%2 (1024 of
2048 rows). Each core redundantly computes the K/V projections for its
batch (cores sharing a batch compute the same k/v) -- no collectives.

All matmuls run in float32r (TF32-like precision, ~4x faster than fp32 on
the PE). Layout strategy:
  - host pre-tiles every input into the exact SBUF layout so all DMAs are
    contiguous per partition
  - k/q projections produce transposed outputs [out_feat, tok]
  - v projection produces natural [tok, feat] with an interleaved ones
    column per head; k^T and v round-trip through DRAM and are re-streamed
    per head-pair during attention
  - scores computed transposed [k_tok, q_tok]; softmax skips the max
    subtraction (scores are O(5) by construction); exp on ACT with the
    1/sqrt(dk) scale folded in
  - attn @ v computed as v^T @ attn with the ones column producing the
    softmax denominators in psum row 64 (M=65)
  - normalization (reciprocal -> K=1 broadcast matmul -> vector multiply)
    is emitted one head late so the tiny broadcast matmul never blocks the
    in-order PE stream
  - o_proj consumes the accumulated [feat, tok] attention output and writes
    the natural-layout result
"""

import numpy as np

import concourse.bass as bass
import concourse.mybir as mybir
import concourse.tile as tile
from concourse import bacc
from concourse.bass_utils import run_bass_kernel_spmd

F32 = mybir.dt.float32
F32R = mybir.dt.float32r
AF = mybir.ActivationFunctionType

H, DM, DK = 16, 1024, 64
B, L = 4, 2048
TOK = 1024          # query tokens per core
FT = DM // 128      # 8 feature tiles
NCORES = 8
SCALE = 1.0 / np.sqrt(DK)

_cache = {}


def _build_nc():
    nc = bacc.Bacc(None, target_bir_lowering=False)

    # host-pre-tiled inputs (see _make_in_maps for element order)
    qt = nc.dram_tensor("qt", [2, 128, FT, 512], F32R, kind="ExternalInput")
    kt = nc.dram_tensor("kt", [4, 128, FT, 512], F32R, kind="ExternalInput")
    vt = nc.dram_tensor("vt", [16, 128, FT, 128], F32R, kind="ExternalInput")
    wkt = nc.dram_tensor("wkt", [128, FT, DM], F32R, kind="ExternalInput")
    wvt = nc.dram_tensor("wvt", [128, FT, DM], F32R, kind="ExternalInput")
    wqt = nc.dram_tensor("wqt", [128, FT, DM], F32R, kind="ExternalInput")
    wot = nc.dram_tensor("wot", [128, FT, DM], F32R, kind="ExternalInput")
    bk_d = nc.dram_tensor("bk", [DM], F32, kind="ExternalInput")
    bq_d = nc.dram_tensor("bq", [DM], F32, kind="ExternalInput")
    bv_d = nc.dram_tensor("bv", [DM], F32, kind="ExternalInput")
    bo_d = nc.dram_tensor("bo", [DM], F32, kind="ExternalInput")
    out_d = nc.dram_tensor("out", [TOK, DM], F32, kind="ExternalOutput")

    def bcast_ap(dram_ap, parts=128):
        return bass.AP(
            tensor=dram_ap.tensor,
            offset=dram_ap.offset,
            ap=[[0, parts]] + [list(d) for d in dram_ap.ap],
        )

    from contextlib import ExitStack

    with tile.TileContext(nc) as tc, ExitStack() as ctx:
        dram = ctx.enter_context(tc.tile_pool(name="dram", bufs=1, space="DRAM"))
        # kT_dram[pair, p, t]: head pair's k^T (two heads stacked on partitions)
        kT_dram = dram.tile([8, 128, L], F32R, name="kT_dram")

        persist = ctx.enter_context(tc.tile_pool(name="persist", bufs=1))
        bk_sb = persist.tile([128, FT], F32, name="bk_sb")
        nc.sync.dma_start(bk_sb[:], bk_d[:].rearrange("(f p) -> p f", p=128))
        bq_sb = persist.tile([128, FT], F32, name="bq_sb")
        nc.sync.dma_start(bq_sb[:], bq_d[:].rearrange("(f p) -> p f", p=128))
        bv_rep = persist.tile([128, DM], F32, name="bv_rep")
        nc.gpsimd.dma_start(bv_rep[:], bcast_ap(bv_d[:]))
        bo_rep = persist.tile([128, DM], F32, name="bo_rep")
        nc.gpsimd.dma_start(bo_rep[:], bcast_ap(bo_d[:]))
        ones_f32 = persist.tile([65, DK], F32, name="ones_f32")
        nc.vector.memset(ones_f32[:], 1.0)
        ones_t = persist.tile([65, DK], F32R, name="ones_t")
        nc.vector.tensor_copy(ones_t[:], ones_f32[:])
        ones_col = persist.tile([128, H, 1], F32, name="ones_col")
        nc.vector.memset(ones_col[:], 1.0)
        zeros_sb = persist.tile([128, 1024], F32, name="zeros_sb")
        nc.vector.memset(zeros_sb[:], 0.0)
        # v_all[p, kt, h, e]: v values + ones column (e=64), SBUF resident
        v_all = persist.tile([128, 16, H, 65], F32R, name="v_all")

        # ---- Phase 1a: v-proj (into SBUF v_all) ----
        with (
            tc.tile_pool(name="p0w", bufs=1) as wpool,
            tc.tile_pool(name="p0c", bufs=2) as cpool,
            tc.tile_pool(name="psA0", bufs=5, space="PSUM") as psA,
        ):
            wv_sb = wpool.tile([128, FT, DM], F32R, name="wv_sb")
            nc.sync.dma_start(wv_sb[:], wvt[:, :, :])

            # v = (V^T)^T @ Wv^T : lhsT = V^T tile, moving = WvT chunk
            for m in range(16):
                vtc = cpool.tile([128, FT, 128], F32R, name="vtc")
                nc.sync.dma_start(vtc[:], vt[m])
                pss = []
                for n in range(2):
                    pss.append(psA.tile([128, 512], F32, name="psAv", tag="psA"))
                for k in range(FT):
                    for n in range(2):
                        nc.tensor.matmul(
                            pss[n][:],
                            vtc[:, k, :],
                            wv_sb[:, k, n * 512:(n + 1) * 512],
                            start=(k == 0),
                            stop=(k == FT - 1),
                        )
                for n in range(2):
                    nc.vector.tensor_add(
                        v_all[:, m, n * 8:(n + 1) * 8, 0:DK],
                        pss[n][:].rearrange("p (g d) -> p g d", d=DK),
                        bv_rep[:, n * 512:(n + 1) * 512].rearrange(
                            "p (g d) -> p g d", d=DK
                        ),
                    )
                nc.vector.tensor_copy(
                    v_all[:, m, :, DK:DK + 1], ones_col[:]
                )

        # ---- Phase 1b: k-proj ----
        with (
            tc.tile_pool(name="p1w", bufs=1) as wpool,
            tc.tile_pool(name="p1s", bufs=3) as spool,
            tc.tile_pool(name="psA", bufs=5, space="PSUM") as psA,
        ):
            wk_sb = wpool.tile([128, FT, DM], F32R, name="wk_sb")
            nc.sync.dma_start(wk_sb[:], wkt[:, :, :])
            # k^T = (Wk^T)^T @ K^T : lhsT = WkT tile, moving = K^T chunk
            ktc = wpool.tile([128, FT, L], F32R, name="ktc")
            for n in range(4):
                nc.sync.dma_start(ktc[:, :, n * 512:(n + 1) * 512], kt[n])
            for m in range(FT):
                pss = []
                for n in range(4):
                    pss.append(psA.tile([128, 512], F32, name="psA", tag="psA"))
                for k in range(FT):
                    for n in range(4):
                        nc.tensor.matmul(
                            pss[n][:],
                            wk_sb[:, k, m * 128:(m + 1) * 128],
                            ktc[:, k, n * 512:(n + 1) * 512],
                            start=(k == 0),
                            stop=(k == FT - 1),
                        )
                for n in range(4):
                    stg = spool.tile([128, 512], F32R, name="stg")
                    nc.scalar.activation(
                        stg[:], pss[n][:], AF.Identity, bias=bk_sb[:, m:m + 1]
                    )
                    nc.sync.dma_start(
                        kT_dram[m, :, n * 512:(n + 1) * 512], stg[:]
                    )

        # ---- Phase 2: q-proj (transposed out, SBUF resident) ----
        qTpool = ctx.enter_context(tc.tile_pool(name="qTp", bufs=1))
        qT_sb = qTpool.tile([128, FT, TOK], F32R, name="qT_sb")
        with (
            tc.tile_pool(name="p3w", bufs=1) as wq_pool,
            tc.tile_pool(name="psB", bufs=5, space="PSUM") as psB,
        ):
            wq_sb = wq_pool.tile([128, FT, DM], F32R, name="wq_sb")
            nc.sync.dma_start(wq_sb[:], wqt[:, :, :])
            qtc = wq_pool.tile([128, FT, TOK], F32R, name="qtc")
            for n in range(2):
                nc.sync.dma_start(qtc[:, :, n * 512:(n + 1) * 512], qt[n])
            for m in range(FT):
                pss = []
                for n in range(2):
                    pss.append(psB.tile([128, 512], F32, name="psB", tag="psB"))
                for k in range(FT):
                    for n in range(2):
                        nc.tensor.matmul(
                            pss[n][:],
                            wq_sb[:, k, m * 128:(m + 1) * 128],
                            qtc[:, k, n * 512:(n + 1) * 512],
                            start=(k == 0),
                            stop=(k == FT - 1),
                        )
                for n in range(2):
                    nc.scalar.activation(
                        qT_sb[:, m, n * 512:(n + 1) * 512],
                        pss[n][:],
                        AF.Identity,
                        bias=bq_sb[:, m:m + 1],
                    )

        # ---- Phase 3: attention, per head, full-array matmuls ----
        aout_pool = ctx.enter_context(tc.tile_pool(name="aout", bufs=1))
        attn_outT = aout_pool.tile([128, FT, TOK], F32R, name="attn_outT")
        with (
            tc.tile_pool(name="kv", bufs=1) as kv_pool,
            tc.tile_pool(name="at", bufs=3) as at_pool,
            tc.tile_pool(name="nrm", bufs=1) as nrm_pool,
            tc.tile_pool(name="stash", bufs=2) as stash_pool,
            tc.tile_pool(name="ps", bufs=4, space="PSUM") as ps_pool,
        ):
            def emit_norm(stash, h):
                recip = nrm_pool.tile([65, TOK], F32R, name="recip")
                with nc.allow_low_precision(reason="softmax denom reciprocal"):
                    nc.vector.reciprocal(recip[64:65, :], stash[64:65, :])
                bc = ps_pool.tile([128, TOK], F32, name="bc", tag="ps")
                for qh in range(2):
                    nc.tensor.matmul(
                        bc[0:64, qh * 512:(qh + 1) * 512],
                        ones_t[64:65, :],
                        recip[64:65, qh * 512:(qh + 1) * 512],
                        start=True,
                        stop=True,
                    )
                bc_sb = nrm_pool.tile([64, TOK], F32, name="bc_sb")
                nc.vector.tensor_copy(bc_sb[:], bc[0:64, :])
                pair, hh = divmod(h, 2)
                if hh == 0:
                    nc.vector.tensor_mul(
                        attn_outT[0:64, pair, :], stash[0:64, :], bc_sb[:]
                    )
                else:
                    tmp = nrm_pool.tile([64, TOK], F32R, name="tmp")
                    nc.vector.tensor_mul(tmp[:], stash[0:64, :], bc_sb[:])
                    nc.sync.dma_start(attn_outT[64:128, pair, :], tmp[:])

            kT_tiles = []
            v_tiles = []
            for par in range(2):
                kt_t = kv_pool.tile([128, L], F32R, name=f"kTp{par}", tag=f"kTp{par}")
                pad0 = (1 - par) * 64
                nc.vector.tensor_copy(kt_t[pad0:pad0 + 64, 0:1024], zeros_sb[0:64, :])
                nc.vector.tensor_copy(kt_t[pad0:pad0 + 64, 1024:2048], zeros_sb[0:64, :])
                kT_tiles.append(kt_t)
                v_t = kv_pool.tile([128, 16, 128], F32R, name=f"vp{par}", tag=f"vp{par}")
                nc.vector.tensor_copy(
                    v_t[:, :, 65:128],
                    zeros_sb[:, 0:16 * 63].rearrange("p (kt e) -> p kt e", e=63),
                )
                v_tiles.append(v_t)

            pending = None
            for h in range(H):
                par = h % 2
                base = par * 64
                kT_head = kT_tiles[par]
                nc.sync.dma_start(
                    kT_head[base:base + 64, :], kT_dram[h // 2, base:base + 64, :]
                )
                v_head = v_tiles[par]
                nc.vector.tensor_copy(v_head[:, :, 0:65], v_all[:, :, h, :])
                av = ps_pool.tile([128, TOK], F32, name="av", tag="ps")
                for kt_i in range(16):
                    sc = ps_pool.tile([128, TOK], F32, name="sc", tag="ps")
                    for qh in range(2):
                        nc.tensor.matmul(
                            sc[:, qh * 512:(qh + 1) * 512],
                            kT_head[:, kt_i * 128:(kt_i + 1) * 128],
                            qT_sb[:, h // 2, qh * 512:(qh + 1) * 512],
                            start=True,
                            stop=True,
                        )
                    atn = at_pool.tile([128, TOK], F32R, name="atn")
                    nc.scalar.activation(atn[:], sc[:], AF.Exp, scale=SCALE)
                    for qh in range(2):
                        nc.tensor.matmul(
                            av[:, qh * 512:(qh + 1) * 512],
                            v_head[:, kt_i, :],
                            atn[:, qh * 512:(qh + 1) * 512],
                            start=(kt_i == 0),
                            stop=(kt_i == 15),
                        )
                    if kt_i == 6 and pending is not None:
                        emit_norm(*pending)
                        pending = None
                stash = stash_pool.tile([65, TOK], F32, name="stash")
                nc.vector.tensor_copy(stash[:], av[0:65, :])
                pending = (stash, h)
            emit_norm(*pending)

        # ---- Phase 4: o-proj ----
        with (
            tc.tile_pool(name="p5w", bufs=1) as wo_pool,
            tc.tile_pool(name="p5o", bufs=3) as o_pool,
            tc.tile_pool(name="psC", bufs=5, space="PSUM") as psC,
        ):
            wo_sb = wo_pool.tile([128, FT, DM], F32R, name="wo_sb")
            nc.sync.dma_start(wo_sb[:], wot[:, :, :])
            for m in range(FT):
                pss = []
                for n in range(2):
                    pss.append(psC.tile([128, 512], F32, name="psC", tag="psC"))
                for k in range(FT):
                    for n in range(2):
                        nc.tensor.matmul(
                            pss[n][:],
                            attn_outT[:, k, m * 128:(m + 1) * 128],
                            wo_sb[:, k, n * 512:(n + 1) * 512],
                            start=(k == 0),
                            stop=(k == FT - 1),
                        )
                osb = o_pool.tile([128, DM], F32, name="osb")
                for n in range(2):
                    nc.vector.tensor_add(
                        osb[:, n * 512:(n + 1) * 512],
                        pss[n][:],
                        bo_rep[:, n * 512:(n + 1) * 512],
                    )
                nc.sync.dma_start(out_d[m * 128:(m + 1) * 128, :], osb[:])

    nc.compile()
    return nc


def _get_nc():
    if "nc" not in _cache:
        _cache["nc"] = _build_nc()
    return _cache["nc"]


def _tile_w(WT):
    # [in, out] -> [128, FT, out] with element [p, f, o] = WT[f*128+p, o]
    return np.ascontiguousarray(WT.reshape(FT, 128, DM).transpose(1, 0, 2))


def _tile_x(XT, n_chunks, chunk):
    # XT [feat, tok] -> [n_chunks, 128, FT, chunk], [n,p,f,t] = XT[f*128+p, n*chunk+t]
    ntok = n_chunks * chunk
    assert XT.shape == (DM, ntok)
    return np.ascontiguousarray(
        XT.reshape(FT, 128, n_chunks, chunk).transpose(2, 1, 0, 3)
    )


def _make_in_maps(V, K, Q, Wv, bv, Wk, bk, Wq, bq, Wo, bo):
    f32 = np.float32
    WqT = _tile_w(np.asarray(Wq, f32).T)
    WkT = _tile_w(np.asarray(Wk, f32).T)
    WvT = _tile_w(np.asarray(Wv, f32).T)
    WoT = _tile_w(np.asarray(Wo, f32).T)
    bq = np.ascontiguousarray(bq, dtype=f32)
    bk = np.ascontiguousarray(bk, dtype=f32)
    bv = np.ascontiguousarray(bv, dtype=f32)
    bo = np.ascontiguousarray(bo, dtype=f32)
    KTs = [_tile_x(np.asarray(K[b], f32).T, 4, 512) for b in range(B)]
    VTs = [_tile_x(np.asarray(V[b], f32).T, 16, 128) for b in range(B)]
    in_maps = []
    for c in range(NCORES):
        b, half = divmod(c, 2)
        qt = _tile_x(
            np.ascontiguousarray(np.asarray(Q[b], f32)[half * TOK:(half + 1) * TOK].T),
            2, 512,
        )
        in_maps.append({
            "qt": qt, "kt": KTs[b], "vt": VTs[b],
            "wqt": WqT, "wkt": WkT, "wvt": WvT, "wot": WoT,
            "bq": bq, "bk": bk, "bv": bv, "bo": bo,
        })
    return in_maps


def _run(V, K, Q, Wv, bv, Wk, bk, Wq, bq, Wo, bo, **run_kwargs):
    nc = _get_nc()
    in_maps = _make_in_maps(V, K, Q, Wv, bv, Wk, bk, Wq, bq, Wo, bo)
    res = run_bass_kernel_spmd(nc, in_maps, core_ids=list(range(NCORES)), **run_kwargs)
    out = np.empty((B, L, DM), np.float32)
    for c, r in enumerate(res.results):
        b, half = divmod(c, 2)
        out[b, half * TOK:(half + 1) * TOK, :] = r["out"]
    return out, res


def kernel(V, K, Q, Wv, bv, Wk, bk, Wq, bq, Wo, bo):
    out, _ = _run(V, K, Q, Wv, bv, Wk, bk, Wq, bq, Wo, bo)
    return out


# revision 14
# speedup vs baseline: 1.6834x; 1.1719x over previous
"""Multi-head attention Trainium2 kernel (8 NeuronCores, SPMD).

Sharding: core c handles batch b = c//2 and query-token half c%2 (1024 of
2048 rows). Each core redundantly computes the K/V projections for its
batch (cores sharing a batch compute the same k/v) -- no collectives.

All matmuls run in float32r (TF32-like precision, ~4x faster than fp32 on
the PE). Layout strategy:
  - host pre-tiles every input into the exact SBUF layout so all DMAs are
    contiguous per partition
  - k/q projections produce transposed outputs [out_feat, tok]
  - v projection produces natural [tok, feat] with an interleaved ones
    column per head; k^T and v round-trip through DRAM and are re-streamed
    per head-pair during attention
  - scores computed transposed [k_tok, q_tok]; softmax skips the max
    subtraction (scores are O(5) by construction); exp on ACT with the
    1/sqrt(dk) scale folded in
  - attn @ v computed as v^T @ attn with the ones column producing the
    softmax denominators in psum row 64 (M=65)
  - normalization (reciprocal -> K=1 broadcast matmul -> vector multiply)
    is emitted one head late so the tiny broadcast matmul never blocks the
    in-order PE stream
  - o_proj consumes the accumulated [feat, tok] attention output and writes
    the natural-layout result
"""

import numpy as np

import concourse.bass as bass
import concourse.mybir as mybir
import concourse.tile as tile
from concourse import bacc
from concourse.bass_utils import run_bass_kernel_spmd

F32 = mybir.dt.float32
F32R = mybir.dt.float32r
AF = mybir.ActivationFunctionType

H, DM, DK = 16, 1024, 64
B, L = 4, 2048
TOK = 1024          # query tokens per core
FT = DM // 128      # 8 feature tiles
NCORES = 8
SCALE = 1.0 / np.sqrt(DK)

_cache = {}


def _build_nc():
    nc = bacc.Bacc(None, target_bir_lowering=False)

    # host-pre-tiled inputs (see _make_in_maps for element order)
    qt = nc.dram_tensor("qt", [2, 128, FT, 512], F32R, kind="ExternalInput")
    kt = nc.dram_tensor("kt", [4, 128, FT, 512], F32R, kind="ExternalInput")
    vt = nc.dram_tensor("vt", [16, 128, FT, 128], F32R, kind="ExternalInput")
    wkt = nc.dram_tensor("wkt", [128, FT, DM], F32R, kind="ExternalInput")
    wvt = nc.dram_tensor("wvt", [128, FT, DM], F32R, kind="ExternalInput")
    wqt = nc.dram_tensor("wqt", [128, FT, DM], F32R, kind="ExternalInput")
    wot = nc.dram_tensor("wot", [128, FT, DM], F32R, kind="ExternalInput")
    bk_d = nc.dram_tensor("bk", [DM], F32, kind="ExternalInput")
    bq_d = nc.dram_tensor("bq", [DM], F32, kind="ExternalInput")
    bv_d = nc.dram_tensor("bv", [DM], F32, kind="ExternalInput")
    bo_d = nc.dram_tensor("bo", [DM], F32, kind="ExternalInput")
    out_d = nc.dram_tensor("out", [TOK, DM], F32, kind="ExternalOutput")

    def bcast_ap(dram_ap, parts=128):
        return bass.AP(
            tensor=dram_ap.tensor,
            offset=dram_ap.offset,
            ap=[[0, parts]] + [list(d) for d in dram_ap.ap],
        )

    from contextlib import ExitStack

    with tile.TileContext(nc, pool_alloc_mode="queue") as tc, ExitStack() as ctx:
        dram = ctx.enter_context(tc.tile_pool(name="dram", bufs=1, space="DRAM"))
        # kT_dram[pair, p, t]: head pair's k^T (two heads stacked on partitions)
        kT_dram = dram.tile([8, 128, L], F32R, name="kT_dram")

        persist = ctx.enter_context(tc.tile_pool(name="persist", bufs=1))
        bk_sb = persist.tile([128, FT], F32, name="bk_sb")
        nc.sync.dma_start(bk_sb[:], bk_d[:].rearrange("(f p) -> p f", p=128))
        bq_sb = persist.tile([128, FT], F32, name="bq_sb")
        nc.sync.dma_start(bq_sb[:], bq_d[:].rearrange("(f p) -> p f", p=128))
        bv_rep = persist.tile([128, DM], F32, name="bv_rep")
        nc.gpsimd.dma_start(bv_rep[:], bcast_ap(bv_d[:]))
        bo_rep = persist.tile([128, DM], F32, name="bo_rep")
        nc.gpsimd.dma_start(bo_rep[:], bcast_ap(bo_d[:]))
        ones_f32 = persist.tile([65, DK], F32, name="ones_f32")
        nc.vector.memset(ones_f32[:], 1.0)
        ones_t = persist.tile([65, DK], F32R, name="ones_t")
        nc.vector.tensor_copy(ones_t[:], ones_f32[:])
        ones_col = persist.tile([128, H, 1], F32, name="ones_col")
        nc.vector.memset(ones_col[:], 1.0)
        zeros_sb = persist.tile([128, 1024], F32, name="zeros_sb")
        nc.vector.memset(zeros_sb[:], 0.0)
        # v_all[p, kt, h, e]: v values + ones column (e=64), SBUF resident
        v_all = persist.tile([128, 16, H, 65], F32R, name="v_all")

        # ---- Phase 1a: v-proj (into SBUF v_all) ----
        with (
            tc.tile_pool(name="p0w", bufs=1) as wpool,
            tc.tile_pool(name="p0c", bufs=2) as cpool,
            tc.tile_pool(name="psA0", bufs=5, space="PSUM") as psA,
        ):
            wv_sb = wpool.tile([128, FT, DM], F32R, name="wv_sb")
            nc.sync.dma_start(wv_sb[:], wvt[:, :, :])

            # v = (V^T)^T @ Wv^T : lhsT = V^T tile, moving = WvT chunk
            for m in range(16):
                vtc = cpool.tile([128, FT, 128], F32R, name="vtc")
                nc.sync.dma_start(vtc[:], vt[m])
                pss = []
                for n in range(2):
                    pss.append(psA.tile([128, 512], F32, name="psAv", tag="psA"))
                for k in range(FT):
                    for n in range(2):
                        nc.tensor.matmul(
                            pss[n][:],
                            vtc[:, k, :],
                            wv_sb[:, k, n * 512:(n + 1) * 512],
                            start=(k == 0),
                            stop=(k == FT - 1),
                        )
                for n in range(2):
                    nc.vector.tensor_add(
                        v_all[:, m, n * 8:(n + 1) * 8, 0:DK],
                        pss[n][:].rearrange("p (g d) -> p g d", d=DK),
                        bv_rep[:, n * 512:(n + 1) * 512].rearrange(
                            "p (g d) -> p g d", d=DK
                        ),
                    )
                nc.vector.tensor_copy(
                    v_all[:, m, :, DK:DK + 1], ones_col[:]
                )

        # ---- Phase 1b: k-proj ----
        with (
            tc.tile_pool(name="p1w", bufs=1) as wpool,
            tc.tile_pool(name="p1s", bufs=3) as spool,
            tc.tile_pool(name="psA", bufs=5, space="PSUM") as psA,
        ):
            wk_sb = wpool.tile([128, FT, DM], F32R, name="wk_sb")
            nc.sync.dma_start(wk_sb[:], wkt[:, :, :])
            # k^T = (Wk^T)^T @ K^T : lhsT = WkT tile, moving = K^T chunk
            ktc = wpool.tile([128, FT, L], F32R, name="ktc")
            for n in range(4):
                nc.sync.dma_start(ktc[:, :, n * 512:(n + 1) * 512], kt[n])
            for m in range(FT):
                pss = []
                for n in range(4):
                    pss.append(psA.tile([128, 512], F32, name="psA", tag="psA"))
                for k in range(FT):
                    for n in range(4):
                        nc.tensor.matmul(
                            pss[n][:],
                            wk_sb[:, k, m * 128:(m + 1) * 128],
                            ktc[:, k, n * 512:(n + 1) * 512],
                            start=(k == 0),
                            stop=(k == FT - 1),
                        )
                for n in range(4):
                    stg = spool.tile([128, 512], F32R, name="stg")
                    nc.scalar.activation(
                        stg[:], pss[n][:], AF.Identity, bias=bk_sb[:, m:m + 1]
                    )
                    nc.sync.dma_start(
                        kT_dram[m, :, n * 512:(n + 1) * 512], stg[:]
                    )

        # ---- Phase 2: q-proj (transposed out, SBUF resident) ----
        qTpool = ctx.enter_context(tc.tile_pool(name="qTp", bufs=1))
        qT_sb = qTpool.tile([128, FT, TOK], F32R, name="qT_sb")
        with (
            tc.tile_pool(name="p3w", bufs=1) as wq_pool,
            tc.tile_pool(name="psB", bufs=5, space="PSUM") as psB,
        ):
            wq_sb = wq_pool.tile([128, FT, DM], F32R, name="wq_sb")
            nc.sync.dma_start(wq_sb[:], wqt[:, :, :])
            qtc = wq_pool.tile([128, FT, TOK], F32R, name="qtc")
            for n in range(2):
                nc.sync.dma_start(qtc[:, :, n * 512:(n + 1) * 512], qt[n])
            for m in range(FT):
                pss = []
                for n in range(2):
                    pss.append(psB.tile([128, 512], F32, name="psB", tag="psB"))
                for k in range(FT):
                    for n in range(2):
                        nc.tensor.matmul(
                            pss[n][:],
                            wq_sb[:, k, m * 128:(m + 1) * 128],
                            qtc[:, k, n * 512:(n + 1) * 512],
                            start=(k == 0),
                            stop=(k == FT - 1),
                        )
                for n in range(2):
                    nc.scalar.activation(
                        qT_sb[:, m, n * 512:(n + 1) * 512],
                        pss[n][:],
                        AF.Identity,
                        bias=bq_sb[:, m:m + 1],
                    )

        # ---- Phase 3: attention, per head, full-array matmuls ----
        aout_pool = ctx.enter_context(tc.tile_pool(name="aout", bufs=1))
        attn_outT = aout_pool.tile([128, FT, TOK], F32R, name="attn_outT")
        with (
            tc.tile_pool(name="kv", bufs=1) as kv_pool,
            tc.tile_pool(name="at", bufs=3) as at_pool,
            tc.tile_pool(name="nrm", bufs=1) as nrm_pool,
            tc.tile_pool(name="stash", bufs=2) as stash_pool,
            tc.tile_pool(name="ps", bufs=4, space="PSUM") as ps_pool,
        ):
            def emit_norm(stash, h):
                recip = nrm_pool.tile([65, TOK], F32R, name="recip")
                with nc.allow_low_precision(reason="softmax denom reciprocal"):
                    nc.vector.reciprocal(recip[64:65, :], stash[64:65, :])
                bc = ps_pool.tile([128, TOK], F32, name="bc", tag="ps")
                for qh in range(2):
                    nc.tensor.matmul(
                        bc[0:64, qh * 512:(qh + 1) * 512],
                        ones_t[64:65, :],
                        recip[64:65, qh * 512:(qh + 1) * 512],
                        start=True,
                        stop=True,
                    )
                bc_sb = nrm_pool.tile([64, TOK], F32, name="bc_sb")
                nc.vector.tensor_copy(bc_sb[:], bc[0:64, :])
                pair, hh = divmod(h, 2)
                if hh == 0:
                    nc.vector.tensor_mul(
                        attn_outT[0:64, pair, :], stash[0:64, :], bc_sb[:]
                    )
                else:
                    tmp = nrm_pool.tile([64, TOK], F32R, name="tmp")
                    nc.vector.tensor_mul(tmp[:], stash[0:64, :], bc_sb[:])
                    nc.sync.dma_start(attn_outT[64:128, pair, :], tmp[:])

            kT_tiles = []
            v_tiles = []
            for par in range(2):
                kt_t = kv_pool.tile([128, L], F32R, name=f"kTp{par}", tag=f"kTp{par}")
                pad0 = (1 - par) * 64
                nc.vector.tensor_copy(kt_t[pad0:pad0 + 64, 0:1024], zeros_sb[0:64, :])
                nc.vector.tensor_copy(kt_t[pad0:pad0 + 64, 1024:2048], zeros_sb[0:64, :])
                kT_tiles.append(kt_t)
                v_t = kv_pool.tile([128, 16, 128], F32R, name=f"vp{par}", tag=f"vp{par}")
                nc.vector.tensor_copy(
                    v_t[:, :, 65:128],
                    zeros_sb[:, 0:16 * 63].rearrange("p (kt e) -> p kt e", e=63),
                )
                v_tiles.append(v_t)

            def emit_head_loads(h):
                par = h % 2
                base = par * 64
                nc.sync.dma_start(
                    kT_tiles[par][base:base + 64, :],
                    kT_dram[h // 2, base:base + 64, :],
                )
                nc.vector.tensor_copy(
                    v_tiles[par][:, :, 0:65], v_all[:, :, h, :]
                )

            emit_head_loads(0)
            emit_head_loads(1)
            pending = None
            for h in range(H):
                par = h % 2
                kT_head = kT_tiles[par]
                v_head = v_tiles[par]
                av = ps_pool.tile([128, TOK], F32, name="av", tag="ps")
                for kt_i in range(16):
                    sc = ps_pool.tile([128, TOK], F32, name="sc", tag="ps")
                    for qh in range(2):
                        nc.tensor.matmul(
                            sc[:, qh * 512:(qh + 1) * 512],
                            kT_head[:, kt_i * 128:(kt_i + 1) * 128],
                            qT_sb[:, h // 2, qh * 512:(qh + 1) * 512],
                            start=True,
                            stop=True,
                        )
                    atn = at_pool.tile([128, TOK], F32R, name="atn")
                    nc.scalar.activation(atn[:], sc[:], AF.Exp, scale=SCALE)
                    for qh in range(2):
                        nc.tensor.matmul(
                            av[:, qh * 512:(qh + 1) * 512],
                            v_head[:, kt_i, :],
                            atn[:, qh * 512:(qh + 1) * 512],
                            start=(kt_i == 0),
                            stop=(kt_i == 15),
                        )
                    if kt_i == 6 and pending is not None:
                        emit_norm(*pending)
                        pending = None
                if h + 2 < H:
                    emit_head_loads(h + 2)
                stash = stash_pool.tile([65, TOK], F32, name="stash")
                nc.vector.tensor_copy(stash[:], av[0:65, :])
                pending = (stash, h)
            emit_norm(*pending)

        # ---- Phase 4: o-proj ----
        with (
            tc.tile_pool(name="p5w", bufs=1) as wo_pool,
            tc.tile_pool(name="p5o", bufs=3) as o_pool,
            tc.tile_pool(name="psC", bufs=5, space="PSUM") as psC,
        ):
            wo_sb = wo_pool.tile([128, FT, DM], F32R, name="wo_sb")
            nc.sync.dma_start(wo_sb[:], wot[:, :, :])
            for m in range(FT):
                pss = []
                for n in range(2):
                    pss.append(psC.tile([128, 512], F32, name="psC", tag="psC"))
                for k in range(FT):
                    for n in range(2):
                        nc.tensor.matmul(
                            pss[n][:],
                            attn_outT[:, k, m * 128:(m + 1) * 128],
                            wo_sb[:, k, n * 512:(n + 1) * 512],
                            start=(k == 0),
                            stop=(k == FT - 1),
                        )
                osb = o_pool.tile([128, DM], F32, name="osb")
                for n in range(2):
                    nc.vector.tensor_add(
                        osb[:, n * 512:(n + 1) * 512],
                        pss[n][:],
                        bo_rep[:, n * 512:(n + 1) * 512],
                    )
                nc.sync.dma_start(out_d[m * 128:(m + 1) * 128, :], osb[:])

    nc.compile()
    return nc


def _get_nc():
    if "nc" not in _cache:
        _cache["nc"] = _build_nc()
    return _cache["nc"]


def _tile_w(WT):
    # [in, out] -> [128, FT, out] with element [p, f, o] = WT[f*128+p, o]
    return np.ascontiguousarray(WT.reshape(FT, 128, DM).transpose(1, 0, 2))


def _tile_x(XT, n_chunks, chunk):
    # XT [feat, tok] -> [n_chunks, 128, FT, chunk], [n,p,f,t] = XT[f*128+p, n*chunk+t]
    ntok = n_chunks * chunk
    assert XT.shape == (DM, ntok)
    return np.ascontiguousarray(
        XT.reshape(FT, 128, n_chunks, chunk).transpose(2, 1, 0, 3)
    )


def _make_in_maps(V, K, Q, Wv, bv, Wk, bk, Wq, bq, Wo, bo):
    f32 = np.float32
    WqT = _tile_w(np.asarray(Wq, f32).T)
    WkT = _tile_w(np.asarray(Wk, f32).T)
    WvT = _tile_w(np.asarray(Wv, f32).T)
    WoT = _tile_w(np.asarray(Wo, f32).T)
    bq = np.ascontiguousarray(bq, dtype=f32)
    bk = np.ascontiguousarray(bk, dtype=f32)
    bv = np.ascontiguousarray(bv, dtype=f32)
    bo = np.ascontiguousarray(bo, dtype=f32)
    KTs = [_tile_x(np.asarray(K[b], f32).T, 4, 512) for b in range(B)]
    VTs = [_tile_x(np.asarray(V[b], f32).T, 16, 128) for b in range(B)]
    in_maps = []
    for c in range(NCORES):
        b, half = divmod(c, 2)
        qt = _tile_x(
            np.ascontiguousarray(np.asarray(Q[b], f32)[half * TOK:(half + 1) * TOK].T),
            2, 512,
        )
        in_maps.append({
            "qt": qt, "kt": KTs[b], "vt": VTs[b],
            "wqt": WqT, "wkt": WkT, "wvt": WvT, "wot": WoT,
            "bq": bq, "bk": bk, "bv": bv, "bo": bo,
        })
    return in_maps


def _run(V, K, Q, Wv, bv, Wk, bk, Wq, bq, Wo, bo, **run_kwargs):
    nc = _get_nc()
    in_maps = _make_in_maps(V, K, Q, Wv, bv, Wk, bk, Wq, bq, Wo, bo)
    res = run_bass_kernel_spmd(nc, in_maps, core_ids=list(range(NCORES)), **run_kwargs)
    out = np.empty((B, L, DM), np.float32)
    for c, r in enumerate(res.results):
        b, half = divmod(c, 2)
        out[b, half * TOK:(half + 1) * TOK, :] = r["out"]
    return out, res


def kernel(V, K, Q, Wv, bv, Wk, bk, Wq, bq, Wo, bo):
    out, _ = _run(V, K, Q, Wv, bv, Wk, bk, Wq, bq, Wo, bo)
    return out
